# revision 24
# baseline (speedup 1.0000x reference)
"""Bounding-box kernel for Trainium2 (Bass/Tile), 8-core SPMD.

Problem: mask [128, 1, 512, 512] f32 -> bbox [128, 4] int32
  (y_min, x_min, y_max, x_max) of the region where mask >= 0.5,
  with (0, 0, H, W) when a row/col has no hit.

Strategy (per core, 16 images):
  - Stream each image [512, 512] as a [128, 4, 512] tile (partition p
    holds 4 contiguous HBM rows -> 8KB DMA descriptors). Images 0-1 are
    triggered from the scalar engine's HWDGE rings (its sequencer is
    ready ~2.5us before sync's), the rest from sync.
  - ACT computes h = Relu(x*2^25 + (1 - 2^24)) -> bf16, exactly 0 iff
    x < 0.5 and >= 1 otherwise.
  - Columns: one-hot lhsT matmuls accumulate per-image column hit-mass
    into PSUM cnt [16, 512] (partition = image). One gpsimd pre-add
    halves one matmul per image; engine loads stay below the 2.64us/image
    DMA cadence with margin even when the clock throttles.
  - Rows: row hit-mass via a TT-max tree on h (tensor_tensor runs at 2
    elems/cycle for 16-bit, tensor_reduce only at 1) -> rowmax fp16;
    threshold {0,1}, PE-transpose into PSUM trow [16, 512] bf16.
  - Extents + fixup use custom DVE ops (registered below):
      EXTENT_MAX: accum_out = max_k (in0 >= 0.5 ? in1 : 0), reading cnt
        (f32 PSUM) / trow (bf16 PSUM) directly against index-const
        tensors; encodes lo as 512-lo so all four extents are max-folds.
      FIX_HI: hi + (hi == 0)*512;  FIX_LO: relu(512 - lo' - (hi == 0)*512)
  - Image 14 is loaded in halves and image 15 in quarters so the
    end-of-stream compute tail is short.
"""

import numpy as np
import ml_dtypes
from contextlib import ExitStack

import concourse.bass as bass
import concourse.bacc as bacc
import concourse.tile as tile
import concourse.mybir as mybir
from concourse.bass_utils import run_bass_kernel_spmd

# --- custom DVE ops -------------------------------------------------------
import concourse.dve_ops as _dve_ops
from concourse.dve_ops import (
    DveOp, OPS, DveOpSpec, _CUSTOM_DVE_ROW_BASE, _SUB_OPCODE_FOR_NAME,
)
from concourse.dve_spec import (
    Spec, Src0, Src1, C0, C1, Zero, select, relu, eq, lower, AluOp,
)


def _ref_extent(in0, in1, s0, s1, imm2):
    b = np.where(in0.astype(np.float32) >= s0, in1.astype(np.float32), 0.0)
    return b, b.reshape(b.shape[0], -1).max(axis=-1, keepdims=True)


def _ref_fixhi(in0, in1, s0, s1, imm2):
    x = in0.astype(np.float32)
    return x + np.where(x == 0.0, s0, 0.0)


def _ref_fixlo(in0, in1, s0, s1, imm2):
    x = in0.astype(np.float32)
    h = in1.astype(np.float32)
    return np.maximum(s1 - x - np.where(h == 0.0, s0, 0.0), 0.0)


EXTENT_MAX = DveOp(
    "EXTENT_MAX",
    Spec(body=select(Src0 >= C0, Src1, Zero), accum=AluOp.MAX,
         reference=_ref_extent),
    subdim=False, uops_sha={},
)
FIX_HI = DveOp(
    "FIX_HI",
    Spec(body=Src0 + select(eq(Src0, Zero), C0, Zero), reference=_ref_fixhi),
    subdim=False, uops_sha={},
)
FIX_LO = DveOp(
    "FIX_LO",
    Spec(body=relu(C1 - Src0 - select(eq(Src1, Zero), C0, Zero)),
         reference=_ref_fixlo),
    subdim=False, uops_sha={},
)


def _register_ops():
    for op in (EXTENT_MAX, FIX_HI, FIX_LO):
        if op.name in _SUB_OPCODE_FOR_NAME:
            continue
        OPS.append(op)
        opcode = _CUSTOM_DVE_ROW_BASE + len(OPS) - 1
        _SUB_OPCODE_FOR_NAME[op.name] = opcode
        for ver in ("v3", "v4"):
            op.uops_sha[ver] = DveOpSpec(
                name=op.name, opcode=opcode, uops=lower(op.spec, ver=ver),
            ).sha(ver)


_register_ops()
# --------------------------------------------------------------------------

N_CORES = 8
N, H, W = 128, 512, 512
NPC = N // N_CORES          # images per core = 16
P = 128                     # SBUF partitions
NBLK = H // P               # 4 row blocks per image
F32 = mybir.dt.float32
BF16 = mybir.dt.bfloat16
FP16 = mybir.dt.float16
I32 = mybir.dt.int32

# Relu(x * 2^25 - (2^24 - 1)) == 0 iff x < 0.5, >= 1 iff x >= 0.5, exact
# for EVERY f32 x (power-of-2 scale is exact; rounding is monotone).
ACT_SCALE = float(2**25)
ACT_BIAS = float(1 - 2**24)

TRACE = False               # test.py sets True to capture a HW profile
LAST_RESULTS = None         # BassKernelResults of the last run

_compiled = None


def _build_nc():
    nc = bacc.Bacc(
        "TRN2", target_bir_lowering=False, debug=False, num_devices=N_CORES
    )
    mask_d = nc.dram_tensor("mask", [NPC * H, W], F32, kind="ExternalInput").ap()
    oneh_d = nc.dram_tensor("onehot", [P, NPC * NPC], BF16, kind="ExternalInput").ap()
    ident_d = nc.dram_tensor("ident", [P, P], BF16, kind="ExternalInput").ap()
    xpack_d = nc.dram_tensor("xpack", [NPC, 2 * W], F32, kind="ExternalInput").ap()
    ypack_d = nc.dram_tensor("ypack", [NPC, 2 * H], F32, kind="ExternalInput").ap()
    bbox_d = nc.dram_tensor("bbox", [NPC, 4], I32, kind="ExternalOutput").ap()

    with tile.TileContext(nc) as tc, ExitStack() as ctx:
        consts = ctx.enter_context(tc.tile_pool(name="consts", bufs=1))
        xpool = ctx.enter_context(tc.tile_pool(name="x", bufs=14))
        hpool = ctx.enter_context(tc.tile_pool(name="h", bufs=10))
        hspool = ctx.enter_context(tc.tile_pool(name="hs", bufs=4))
        tpool = ctx.enter_context(tc.tile_pool(name="t", bufs=3))
        lasth = ctx.enter_context(tc.tile_pool(name="lasth", bufs=2))
        lastq = ctx.enter_context(tc.tile_pool(name="lastq", bufs=4))
        small = ctx.enter_context(tc.tile_pool(name="small", bufs=1))
        scratch = ctx.enter_context(tc.tile_pool(name="scr", bufs=2))
        psum = ctx.enter_context(tc.tile_pool(name="psum", bufs=1, space="PSUM"))

        oneh = consts.tile([P, NPC * NPC], BF16)
        ident = consts.tile([P, P], BF16)
        xpack = consts.tile([NPC, 2, W], F32)
        ypack = consts.tile([NPC, 2, H], F32)
        act_bias = consts.tile([P, 1], F32)

        # rowmax col 4*i + b = row hit-mass max of image i, sub-row b
        # (partition p, block b <-> image row h = 4p + b)
        rowmax = small.tile([P, NPC * NBLK], FP16)
        rowhitB = small.tile([P, NBLK * NPC], BF16)   # b-major, {0, 1}
        cnt_ps = psum.tile([NPC, W], F32)    # per-image column hit-mass
        trow_ps = psum.tile([NPC, H], BF16)  # per-image row hits {0, 1}

        # --- startup: first images via scalar rings ------------------
        # The scalar sequencer issues DMAs ~2.5us before sync's preamble
        # finishes. Only image 0 goes on the scalar rings: it finishes
        # streaming before sync's first image even starts, so the in-order
        # RELU chain starts ~4us earlier. Giving scalar more images backfires
        # -- the DMA engines round-robin between ring sets, so early images
        # would share bandwidth with sync's prefetch of later ones and land
        # last (priority inversion on the in-order consumer).
        N_SCALAR_IMGS = 1
        x01 = []
        for i in range(N_SCALAR_IMGS):
            x = xpool.tile([P, NBLK, W], F32, tag="x")
            nc.scalar.dma_start(
                out=x[:],
                in_=mask_d[i * H:(i + 1) * H, :].rearrange("(p b) w -> p b w", p=P),
            )
            x01.append(x)
        nc.scalar.dma_start(out=oneh[:], in_=oneh_d)
        nc.vector.memset(act_bias[:], ACT_BIAS)

        def image_compute(i, x):
            # (gpsimd tensor_scalar is ~7x slower than its ADD fast path
            # -- ucode-emulated -- so all thresholding stays on ACT)
            h = hpool.tile([P, NBLK, W], BF16, tag="h")
            nc.scalar.activation(
                h[:], x[:], mybir.ActivationFunctionType.Relu,
                bias=act_bias[:], scale=ACT_SCALE,
            )
            # row hit-mass: TT-max tree (2x mode) + short 1x reduce
            t1 = tpool.tile([P, NBLK, W // 2], BF16, tag="t1")
            nc.vector.tensor_max(t1[:], h[:, :, 0:W // 2], h[:, :, W // 2:W])
            t2 = tpool.tile([P, NBLK, W // 4], BF16, tag="t2")
            nc.vector.tensor_max(t2[:], t1[:, :, 0:W // 4], t1[:, :, W // 4:W // 2])
            nc.vector.tensor_reduce(
                out=rowmax[:, 4 * i:4 * i + 4], in_=t2[:],
                axis=mybir.AxisListType.X, op=mybir.AluOpType.max,
            )
            lhsT = oneh[:, i * NPC:(i + 1) * NPC]
            # one gpsimd pre-add per image; PE takes blocks 2,3 directly
            hs = hspool.tile([P, W], BF16)
            nc.gpsimd.tensor_add(hs[:], h[:, 0, :], h[:, 1, :])
            nc.tensor.matmul(cnt_ps[:, :], lhsT, hs[:],
                             start=(i == 0), stop=False)
            nc.tensor.matmul(cnt_ps[:, :], lhsT, h[:, 2, :],
                             start=False, stop=False)
            nc.tensor.matmul(cnt_ps[:, :], lhsT, h[:, 3, :],
                             start=False, stop=False)

        for i in range(N_SCALAR_IMGS):
            image_compute(i, x01[i])

        for i in range(N_SCALAR_IMGS, NPC - 2):
            x = xpool.tile([P, NBLK, W], F32, tag="x")
            nc.sync.dma_start(
                out=x[:],
                in_=mask_d[i * H:(i + 1) * H, :].rearrange("(p b) w -> p b w", p=P),
            )
            image_compute(i, x)
            if i == NPC - 4:
                # tail consts: late enough not to delay the mask stream
                # start, early enough to land well before the tail
                nc.sync.dma_start(out=ident[:], in_=ident_d)
                nc.sync.dma_start(
                    out=xpack[:], in_=xpack_d.rearrange("p (a w) -> p a w", a=2))
                nc.sync.dma_start(
                    out=ypack[:], in_=ypack_d.rearrange("p (a w) -> p a w", a=2))

        # images 14-15 take a short-latency row path: row maxes straight
        # from x (f32, exact 0.5 threshold, no RELU dependency) so the
        # Vector chain is never blocked behind ACT at the stream end.
        rowmaxF = small.tile([P, 2 * NBLK], F32)   # col 4*(i-14) + b

        # image 14: two halves (no gpsimd hop)
        i = NPC - 2
        lhsT = oneh[:, i * NPC:(i + 1) * NPC]
        for u in range(2):
            x = lasth.tile([P, 2, W], F32, tag="xh")
            nc.sync.dma_start(
                out=x[:],
                in_=mask_d[i * H:(i + 1) * H, :]
                .rearrange("(p b) w -> p b w", p=P)[:, 2 * u:2 * u + 2, :],
            )
            h = lasth.tile([P, 2, W], BF16, tag="hh")
            nc.scalar.activation(
                h[:], x[:], mybir.ActivationFunctionType.Relu,
                bias=act_bias[:], scale=ACT_SCALE,
            )
            nc.vector.tensor_reduce(
                out=rowmaxF[:, 2 * u:2 * u + 2], in_=x[:],
                axis=mybir.AxisListType.X, op=mybir.AluOpType.max,
            )
            for b in range(2):
                nc.tensor.matmul(cnt_ps[:, :], lhsT, h[:, b, :],
                                 start=False, stop=False)

        # rowhitB for images 0..13 (b-major out) while images 14-15 stream
        rowhit_iview = rowhitB.rearrange("p (b i) -> p i b", b=NBLK)
        nc.vector.tensor_scalar(
            rowhit_iview[:, 0:NPC - 2, :],
            rowmax[:, 0:4 * (NPC - 2)].rearrange("p (i b) -> p i b", b=NBLK),
            0.5, None, mybir.AluOpType.is_ge,
        )

        # image 15: four quarters. Row-maxes come straight from x (f32 --
        # exact 0.5 threshold) so the rows path has no RELU dependency and
        # starts the moment each quarter lands; ACT's h is only needed for
        # the column matmuls. The PE transposes are emitted BEFORE the
        # quarter matmuls so trow is ready while the cnt accumulation is
        # still finishing.
        i = NPC - 1
        lhsT = oneh[:, i * NPC:(i + 1) * NPC]
        for u in range(4):
            x = lastq.tile([P, 1, W], F32, tag="xq")
            nc.sync.dma_start(
                out=x[:],
                in_=mask_d[i * H:(i + 1) * H, :]
                .rearrange("(p b) w -> p b w", p=P)[:, u:u + 1, :],
            )
            h = lastq.tile([P, 1, W], BF16, tag="hq")
            nc.scalar.activation(
                h[:], x[:], mybir.ActivationFunctionType.Relu,
                bias=act_bias[:], scale=ACT_SCALE,
            )
            nc.vector.tensor_reduce(
                out=rowmaxF[:, NBLK + u:NBLK + u + 1], in_=x[:, 0, :],
                axis=mybir.AxisListType.X, op=mybir.AluOpType.max,
            )
            nc.tensor.matmul(cnt_ps[:, :], lhsT, h[:, 0, :],
                             start=False, stop=(u == 3))

        # --- tail ----------------------------------------------------
        nc.vector.tensor_scalar(
            rowhit_iview[:, NPC - 2:NPC, :],
            rowmaxF[:].rearrange("p (i b) -> p i b", b=NBLK),
            0.5, None, mybir.AluOpType.is_ge,
        )
        rowhitB_v = rowhitB.rearrange("p (b i) -> p b i", b=NBLK)
        for b in range(NBLK):
            nc.tensor.matmul(
                trow_ps[:, b * P:(b + 1) * P], rowhitB_v[:, b, :], ident[:],
                is_transpose=True, start=True, stop=True,
            )

        # ext = (ylo', xlo', yhi, xhi); lo' = 512 - lo for hits, 0 if none
        ext = small.tile([NPC, 4], F32)

        def extent(in0, in1, out_col):
            scr = scratch.tile([NPC, W], F32, tag="scr")
            nc.vector._custom_dve(
                EXTENT_MAX, out=scr[:], in0=in0, in1=in1,
                s0=0.5, accum_out=ext[:, out_col:out_col + 1],
            )

        # rows first: trow is ready before the last quarter's cnt matmul
        extent(trow_ps[:], ypack[:, 0, :], 0)  # ylo'
        extent(trow_ps[:], ypack[:, 1, :], 2)  # yhi
        extent(cnt_ps[:], xpack[:, 0, :], 1)   # xlo'
        extent(cnt_ps[:], xpack[:, 1, :], 3)   # xhi

        # fixup: bbox = (ymin, xmin, ymax, xmax)
        bbox_f = small.tile([NPC, 4], F32)
        nc.vector._custom_dve(
            FIX_HI, out=bbox_f[:, 2:4], in0=ext[:, 2:4], s0=float(H),
        )
        nc.vector._custom_dve(
            FIX_LO, out=bbox_f[:, 0:2], in0=ext[:, 0:2], in1=ext[:, 2:4],
            s0=float(H), s1=float(H),
        )
        bbox_i = small.tile([NPC, 4], I32)
        nc.vector.tensor_copy(bbox_i[:], bbox_f[:])
        nc.scalar.dma_start(out=bbox_d, in_=bbox_i[:])

    nc.compile()
    return nc


def _consts():
    oneh = np.zeros((P, NPC * NPC), dtype=ml_dtypes.bfloat16)
    for i in range(NPC):
        oneh[:, i * NPC + i] = 1.0
    ident = np.eye(P, dtype=ml_dtypes.bfloat16)
    f = np.arange(W, dtype=np.float32)
    xpack = np.broadcast_to(
        np.concatenate([W - f, f + 1]), (NPC, 2 * W)
    ).astype(np.float32)
    # trow free index f maps to image row h = 4*(f % 128) + f // 128
    fi = np.arange(H)
    hperm = (4 * (fi % P) + fi // P).astype(np.float32)
    ypack = np.broadcast_to(
        np.concatenate([H - hperm, hperm + 1]), (NPC, 2 * H)
    ).astype(np.float32)
    return oneh, ident, xpack, ypack


def kernel(mask):
    global _compiled, LAST_RESULTS
    mask = np.ascontiguousarray(np.asarray(mask), dtype=np.float32)
    assert mask.shape == (N, 1, H, W), mask.shape
    if _compiled is None:
        _compiled = _build_nc()
    nc = _compiled
    oneh, ident, xpack, ypack = _consts()
    m = mask.reshape(N, H, W)
    in_maps = []
    for c in range(N_CORES):
        in_maps.append({
            "mask": np.ascontiguousarray(
                m[c * NPC:(c + 1) * NPC].reshape(NPC * H, W)
            ),
            "onehot": oneh,
            "ident": ident,
            "xpack": xpack,
            "ypack": ypack,
        })
    res = run_bass_kernel_spmd(nc, in_maps, list(range(N_CORES)), trace=TRACE)
    LAST_RESULTS = res
    out = np.concatenate([res.results[c]["bbox"] for c in range(N_CORES)], axis=0)
    return out.astype(np.int32, copy=False)


# revision 25
# speedup vs baseline: 1.1819x; 1.1819x over previous
"""Bounding-box kernel for Trainium2 (Bass/Tile), 8-core SPMD.

Problem: mask [128, 1, 512, 512] f32 -> bbox [128, 4] int32
  (y_min, x_min, y_max, x_max) of the region where mask >= 0.5,
  with (0, 0, H, W) when a row/col has no hit.

Strategy (per core, 16 images):
  - Stream each image [512, 512] as a [128, 4, 512] tile (partition p
    holds 4 contiguous HBM rows -> 8KB DMA descriptors). Images 0-1 are
    triggered from the scalar engine's HWDGE rings (its sequencer is
    ready ~2.5us before sync's), the rest from sync.
  - ACT computes h = Relu(x*2^25 + (1 - 2^24)) -> bf16, exactly 0 iff
    x < 0.5 and >= 1 otherwise.
  - Columns: one-hot lhsT matmuls accumulate per-image column hit-mass
    into PSUM cnt [16, 512] (partition = image). One gpsimd pre-add
    halves one matmul per image; engine loads stay below the 2.64us/image
    DMA cadence with margin even when the clock throttles.
  - Rows: row hit-mass via a TT-max tree on h (tensor_tensor runs at 2
    elems/cycle for 16-bit, tensor_reduce only at 1) -> rowmax fp16;
    threshold {0,1}, PE-transpose into PSUM trow [16, 512] bf16.
  - Extents + fixup use custom DVE ops (registered below):
      EXTENT_MAX: accum_out = max_k (in0 >= 0.5 ? in1 : 0), reading cnt
        (f32 PSUM) / trow (bf16 PSUM) directly against index-const
        tensors; encodes lo as 512-lo so all four extents are max-folds.
      FIX_HI: hi + (hi == 0)*512;  FIX_LO: relu(512 - lo' - (hi == 0)*512)
  - Image 14 is loaded in halves and image 15 in quarters so the
    end-of-stream compute tail is short.
"""

import numpy as np
import ml_dtypes
from contextlib import ExitStack

import concourse.bass as bass
import concourse.bacc as bacc
import concourse.tile as tile
import concourse.mybir as mybir
from concourse.bass_utils import run_bass_kernel_spmd

# --- custom DVE ops -------------------------------------------------------
import concourse.dve_ops as _dve_ops
from concourse.dve_ops import (
    DveOp, OPS, DveOpSpec, _CUSTOM_DVE_ROW_BASE, _SUB_OPCODE_FOR_NAME,
)
from concourse.dve_spec import (
    Spec, Src0, Src1, C0, C1, Zero, select, relu, eq, lower, AluOp,
)


def _ref_extent(in0, in1, s0, s1, imm2):
    b = np.where(in0.astype(np.float32) >= s0, in1.astype(np.float32), 0.0)
    return b, b.reshape(b.shape[0], -1).max(axis=-1, keepdims=True)


def _ref_fixhi(in0, in1, s0, s1, imm2):
    x = in0.astype(np.float32)
    return x + np.where(x == 0.0, s0, 0.0)


def _ref_fixlo(in0, in1, s0, s1, imm2):
    x = in0.astype(np.float32)
    h = in1.astype(np.float32)
    return np.maximum(s1 - x - np.where(h == 0.0, s0, 0.0), 0.0)


EXTENT_MAX = DveOp(
    "EXTENT_MAX",
    Spec(body=select(Src0 >= C0, Src1, Zero), accum=AluOp.MAX,
         reference=_ref_extent),
    subdim=False, uops_sha={},
)
FIX_HI = DveOp(
    "FIX_HI",
    Spec(body=Src0 + select(eq(Src0, Zero), C0, Zero), reference=_ref_fixhi),
    subdim=False, uops_sha={},
)
FIX_LO = DveOp(
    "FIX_LO",
    Spec(body=relu(C1 - Src0 - select(eq(Src1, Zero), C0, Zero)),
         reference=_ref_fixlo),
    subdim=False, uops_sha={},
)


def _register_ops():
    for op in (EXTENT_MAX, FIX_HI, FIX_LO):
        if op.name in _SUB_OPCODE_FOR_NAME:
            continue
        OPS.append(op)
        opcode = _CUSTOM_DVE_ROW_BASE + len(OPS) - 1
        _SUB_OPCODE_FOR_NAME[op.name] = opcode
        for ver in ("v3", "v4"):
            op.uops_sha[ver] = DveOpSpec(
                name=op.name, opcode=opcode, uops=lower(op.spec, ver=ver),
            ).sha(ver)


_register_ops()
# --------------------------------------------------------------------------

N_CORES = 8
N, H, W = 128, 512, 512
NPC = N // N_CORES          # images per core = 16
P = 128                     # SBUF partitions
NBLK = H // P               # 4 row blocks per image
F32 = mybir.dt.float32
BF16 = mybir.dt.bfloat16
FP16 = mybir.dt.float16
I32 = mybir.dt.int32

# Relu(x * 2^25 - (2^24 - 1)) == 0 iff x < 0.5, >= 1 iff x >= 0.5, exact
# for EVERY f32 x (power-of-2 scale is exact; rounding is monotone).
ACT_SCALE = float(2**25)
ACT_BIAS = float(1 - 2**24)

TRACE = False               # test.py sets True to capture a HW profile
LAST_RESULTS = None         # BassKernelResults of the last run

_compiled = None


def _build_nc():
    nc = bacc.Bacc(
        "TRN2", target_bir_lowering=False, debug=False, num_devices=N_CORES
    )
    mask_d = nc.dram_tensor("mask", [NPC * H, W], F32, kind="ExternalInput").ap()
    oneh_d = nc.dram_tensor("onehot", [P, NPC * NPC], BF16, kind="ExternalInput").ap()
    ident_d = nc.dram_tensor("ident", [P, P], BF16, kind="ExternalInput").ap()
    xpack_d = nc.dram_tensor("xpack", [NPC, 2 * W], F32, kind="ExternalInput").ap()
    ypack_d = nc.dram_tensor("ypack", [NPC, 2 * H], F32, kind="ExternalInput").ap()
    bbox_d = nc.dram_tensor("bbox", [NPC, 4], I32, kind="ExternalOutput").ap()

    with tile.TileContext(nc) as tc, ExitStack() as ctx:
        consts = ctx.enter_context(tc.tile_pool(name="consts", bufs=1))
        xpool = ctx.enter_context(tc.tile_pool(name="x", bufs=11))
        hpool = ctx.enter_context(tc.tile_pool(name="h", bufs=13))
        hspool = ctx.enter_context(tc.tile_pool(name="hs", bufs=6))
        tpool = ctx.enter_context(tc.tile_pool(name="t", bufs=4))
        lasth = ctx.enter_context(tc.tile_pool(name="lasth", bufs=2))
        lastq = ctx.enter_context(tc.tile_pool(name="lastq", bufs=4))
        small = ctx.enter_context(tc.tile_pool(name="small", bufs=1))
        scratch = ctx.enter_context(tc.tile_pool(name="scr", bufs=2))
        psum = ctx.enter_context(tc.tile_pool(name="psum", bufs=1, space="PSUM"))

        oneh = consts.tile([P, NPC * NPC], BF16)
        ident = consts.tile([P, P], BF16)
        xpack = consts.tile([NPC, 2, W], F32)
        ypack = consts.tile([NPC, 2, H], F32)
        act_bias = consts.tile([P, 1], F32)

        # rowmax col 4*i + b = row hit-mass max of image i, sub-row b
        # (partition p, block b <-> image row h = 4p + b)
        rowmax = small.tile([P, NPC * NBLK], FP16)
        rowhitB = small.tile([P, NBLK * NPC], BF16)   # b-major, {0, 1}
        cnt_ps = psum.tile([NPC, W], F32)    # per-image column hit-mass
        trow_ps = psum.tile([NPC, H], BF16)  # per-image row hits {0, 1}

        # --- startup: first images via scalar rings ------------------
        # The scalar sequencer issues DMAs ~2.5us before sync's preamble
        # finishes. Only image 0 goes on the scalar rings: it finishes
        # streaming before sync's first image even starts, so the in-order
        # RELU chain starts ~4us earlier. Giving scalar more images backfires
        # -- the DMA engines round-robin between ring sets, so early images
        # would share bandwidth with sync's prefetch of later ones and land
        # last (priority inversion on the in-order consumer).
        N_SCALAR_IMGS = 1
        x01 = []
        for i in range(N_SCALAR_IMGS):
            x = xpool.tile([P, NBLK, W], F32, tag="x")
            nc.scalar.dma_start(
                out=x[:],
                in_=mask_d[i * H:(i + 1) * H, :].rearrange("(p b) w -> p b w", p=P),
            )
            x01.append(x)
        nc.scalar.dma_start(out=oneh[:], in_=oneh_d)
        nc.vector.memset(act_bias[:], ACT_BIAS)

        def image_compute(i, x):
            # (gpsimd tensor_scalar is ~7x slower than its ADD fast path
            # -- ucode-emulated -- so all thresholding stays on ACT)
            h = hpool.tile([P, NBLK, W], BF16, tag="h")
            nc.scalar.activation(
                h[:], x[:], mybir.ActivationFunctionType.Relu,
                bias=act_bias[:], scale=ACT_SCALE,
            )
            # row hit-mass: TT-max tree (2x mode) + short 1x reduce
            t1 = tpool.tile([P, NBLK, W // 2], BF16, tag="t1")
            nc.vector.tensor_max(t1[:], h[:, :, 0:W // 2], h[:, :, W // 2:W])
            t2 = tpool.tile([P, NBLK, W // 4], BF16, tag="t2")
            nc.vector.tensor_max(t2[:], t1[:, :, 0:W // 4], t1[:, :, W // 4:W // 2])
            nc.vector.tensor_reduce(
                out=rowmax[:, 4 * i:4 * i + 4], in_=t2[:],
                axis=mybir.AxisListType.X, op=mybir.AluOpType.max,
            )
            lhsT = oneh[:, i * NPC:(i + 1) * NPC]
            # one gpsimd pre-add per image; PE takes blocks 2,3 directly
            hs = hspool.tile([P, W], BF16)
            nc.gpsimd.tensor_add(hs[:], h[:, 0, :], h[:, 1, :])
            nc.tensor.matmul(cnt_ps[:, :], lhsT, hs[:],
                             start=(i == 0), stop=False)
            nc.tensor.matmul(cnt_ps[:, :], lhsT, h[:, 2, :],
                             start=False, stop=False)
            nc.tensor.matmul(cnt_ps[:, :], lhsT, h[:, 3, :],
                             start=False, stop=False)

        for i in range(N_SCALAR_IMGS):
            image_compute(i, x01[i])

        for i in range(N_SCALAR_IMGS, NPC - 2):
            x = xpool.tile([P, NBLK, W], F32, tag="x")
            nc.sync.dma_start(
                out=x[:],
                in_=mask_d[i * H:(i + 1) * H, :].rearrange("(p b) w -> p b w", p=P),
            )
            image_compute(i, x)
            if i == NPC - 4:
                # tail consts: late enough not to delay the mask stream
                # start, early enough to land well before the tail
                nc.sync.dma_start(out=ident[:], in_=ident_d)
                nc.sync.dma_start(
                    out=xpack[:], in_=xpack_d.rearrange("p (a w) -> p a w", a=2))
                nc.sync.dma_start(
                    out=ypack[:], in_=ypack_d.rearrange("p (a w) -> p a w", a=2))

        # images 14-15 take a short-latency row path: row maxes straight
        # from x (f32, exact 0.5 threshold, no RELU dependency) so the
        # Vector chain is never blocked behind ACT at the stream end.
        rowmaxF = small.tile([P, 2 * NBLK], F32)   # col 4*(i-14) + b

        # image 14: two halves (no gpsimd hop)
        i = NPC - 2
        lhsT = oneh[:, i * NPC:(i + 1) * NPC]
        for u in range(2):
            x = lasth.tile([P, 2, W], F32, tag="xh")
            nc.sync.dma_start(
                out=x[:],
                in_=mask_d[i * H:(i + 1) * H, :]
                .rearrange("(p b) w -> p b w", p=P)[:, 2 * u:2 * u + 2, :],
            )
            h = lasth.tile([P, 2, W], BF16, tag="hh")
            nc.scalar.activation(
                h[:], x[:], mybir.ActivationFunctionType.Relu,
                bias=act_bias[:], scale=ACT_SCALE,
            )
            nc.vector.tensor_reduce(
                out=rowmaxF[:, 2 * u:2 * u + 2], in_=x[:],
                axis=mybir.AxisListType.X, op=mybir.AluOpType.max,
            )
            for b in range(2):
                nc.tensor.matmul(cnt_ps[:, :], lhsT, h[:, b, :],
                                 start=False, stop=False)

        # rowhitB for images 0..13 (b-major out) while images 14-15 stream
        rowhit_iview = rowhitB.rearrange("p (b i) -> p i b", b=NBLK)
        nc.vector.tensor_scalar(
            rowhit_iview[:, 0:NPC - 2, :],
            rowmax[:, 0:4 * (NPC - 2)].rearrange("p (i b) -> p i b", b=NBLK),
            0.5, None, mybir.AluOpType.is_ge,
        )

        # image 15: four quarters. Row-maxes come straight from x (f32 --
        # exact 0.5 threshold) so the rows path has no RELU dependency and
        # starts the moment each quarter lands; ACT's h is only needed for
        # the column matmuls. The PE transposes are emitted BEFORE the
        # quarter matmuls so trow is ready while the cnt accumulation is
        # still finishing.
        i = NPC - 1
        lhsT = oneh[:, i * NPC:(i + 1) * NPC]
        for u in range(4):
            x = lastq.tile([P, 1, W], F32, tag="xq")
            nc.sync.dma_start(
                out=x[:],
                in_=mask_d[i * H:(i + 1) * H, :]
                .rearrange("(p b) w -> p b w", p=P)[:, u:u + 1, :],
            )
            h = lastq.tile([P, 1, W], BF16, tag="hq")
            nc.scalar.activation(
                h[:], x[:], mybir.ActivationFunctionType.Relu,
                bias=act_bias[:], scale=ACT_SCALE,
            )
            nc.vector.tensor_reduce(
                out=rowmaxF[:, NBLK + u:NBLK + u + 1], in_=x[:, 0, :],
                axis=mybir.AxisListType.X, op=mybir.AluOpType.max,
            )
            nc.tensor.matmul(cnt_ps[:, :], lhsT, h[:, 0, :],
                             start=False, stop=(u == 3))

        # --- tail ----------------------------------------------------
        nc.vector.tensor_scalar(
            rowhit_iview[:, NPC - 2:NPC, :],
            rowmaxF[:].rearrange("p (i b) -> p i b", b=NBLK),
            0.5, None, mybir.AluOpType.is_ge,
        )
        rowhitB_v = rowhitB.rearrange("p (b i) -> p b i", b=NBLK)
        for b in range(NBLK):
            nc.tensor.matmul(
                trow_ps[:, b * P:(b + 1) * P], rowhitB_v[:, b, :], ident[:],
                is_transpose=True, start=True, stop=True,
            )

        # ext = (ylo', xlo', yhi, xhi); lo' = 512 - lo for hits, 0 if none
        ext = small.tile([NPC, 4], F32)

        def extent(in0, in1, out_col):
            scr = scratch.tile([NPC, W], F32, tag="scr")
            nc.vector._custom_dve(
                EXTENT_MAX, out=scr[:], in0=in0, in1=in1,
                s0=0.5, accum_out=ext[:, out_col:out_col + 1],
            )

        # rows first: trow is ready before the last quarter's cnt matmul
        extent(trow_ps[:], ypack[:, 0, :], 0)  # ylo'
        extent(trow_ps[:], ypack[:, 1, :], 2)  # yhi
        extent(cnt_ps[:], xpack[:, 0, :], 1)   # xlo'
        extent(cnt_ps[:], xpack[:, 1, :], 3)   # xhi

        # fixup: bbox = (ymin, xmin, ymax, xmax)
        bbox_f = small.tile([NPC, 4], F32)
        nc.vector._custom_dve(
            FIX_HI, out=bbox_f[:, 2:4], in0=ext[:, 2:4], s0=float(H),
        )
        nc.vector._custom_dve(
            FIX_LO, out=bbox_f[:, 0:2], in0=ext[:, 0:2], in1=ext[:, 2:4],
            s0=float(H), s1=float(H),
        )
        bbox_i = small.tile([NPC, 4], I32)
        nc.vector.tensor_copy(bbox_i[:], bbox_f[:])
        nc.scalar.dma_start(out=bbox_d, in_=bbox_i[:])

    nc.compile()
    return nc


def _consts():
    oneh = np.zeros((P, NPC * NPC), dtype=ml_dtypes.bfloat16)
    for i in range(NPC):
        oneh[:, i * NPC + i] = 1.0
    ident = np.eye(P, dtype=ml_dtypes.bfloat16)
    f = np.arange(W, dtype=np.float32)
    xpack = np.broadcast_to(
        np.concatenate([W - f, f + 1]), (NPC, 2 * W)
    ).astype(np.float32)
    # trow free index f maps to image row h = 4*(f % 128) + f // 128
    fi = np.arange(H)
    hperm = (4 * (fi % P) + fi // P).astype(np.float32)
    ypack = np.broadcast_to(
        np.concatenate([H - hperm, hperm + 1]), (NPC, 2 * H)
    ).astype(np.float32)
    return oneh, ident, xpack, ypack


def kernel(mask):
    global _compiled, LAST_RESULTS
    mask = np.ascontiguousarray(np.asarray(mask), dtype=np.float32)
    assert mask.shape == (N, 1, H, W), mask.shape
    if _compiled is None:
        _compiled = _build_nc()
    nc = _compiled
    oneh, ident, xpack, ypack = _consts()
    m = mask.reshape(N, H, W)
    in_maps = []
    for c in range(N_CORES):
        in_maps.append({
            "mask": np.ascontiguousarray(
                m[c * NPC:(c + 1) * NPC].reshape(NPC * H, W)
            ),
            "onehot": oneh,
            "ident": ident,
            "xpack": xpack,
            "ypack": ypack,
        })
    res = run_bass_kernel_spmd(nc, in_maps, list(range(N_CORES)), trace=TRACE)
    LAST_RESULTS = res
    out = np.concatenate([res.results[c]["bbox"] for c in range(N_CORES)], axis=0)
    return out.astype(np.int32, copy=False)
